# revision 1
# baseline (speedup 1.0000x reference)
"""Distributed attention kernel for Trainium2 (8 NeuronCores).

Reference computation (B=2, N=2048, C=1024, H=16, D=64, ALPHA=0.5):
    qkv = x @ W_qkv -> q,k,v [B,H,N,D]
    attn = softmax(q @ k^T / sqrt(D))
    attn = 0.5*dm + 0.5*attn
    out  = (attn @ v).reshape(B,N,C) @ W_proj + b_proj

Sharding: 8 cores = 2 batches x 4 head-groups (4 heads each).
Each core computes its head-group's slice end-to-end, including a partial
projection (row-slice of W_proj); host sums the 4 partials per batch.

On-device layout strategy (per core):
  - x arrives transposed [C, N] so the C-contraction has C on partitions.
  - q,k are produced transposed [Dg=256, N] (head-dim on partitions).
  - scores are computed transposed: S^T[k',q] = k^T.T @ q^T, so softmax's
    exp runs on ScalarE straight out of PSUM and the sum-over-k' is folded
    into the attn@v matmul via a ones-column appended to v (lhsT=[m, 65]:
    row 64 of the PSUM accumulator receives sum_m e[m,q] = the softmax
    denominator) -- no extra reduction pass over the N^2 matrix.
  - max-subtraction is skipped: scores are ~N(0,1), |s| < ~8 << 88, so
    exp never overflows in fp32.
  - dm is pre-halved + transposed on host and accumulated through its own
    matmul stream with v as the stationary operand.
  - the ones column holds 2.0, so the accumulator row is 2r and the
    normalization constant 0.5/r is a plain reciprocal.
  - normalization (per q column) is applied after attn@v on the small
    [64, 512] output tiles; the row vector 0.5/r is broadcast across
    partitions via a DRAM-bounce DMA (0-step partition APs are only legal
    on the DRAM side), or via a tiny fp16 PE matmul on the final chunk
    where the PE is idle.
  - all matmul operands are fp16 (1 cycle/row at the PE like bf16, but
    10-bit mantissa); PSUM accumulation stays fp32.
  - scores for a head pair land in one [128,1024] PSUM tile so each exp
    covers two heads (halves ScalarE instruction overhead -- ScalarE exp
    over the N^2 scores is the second-busiest engine after the PE).
"""

import numpy as np

B, N, C, H, D = 2, 2048, 1024, 16, 64
NCORES = 8
HG = 4                # head-groups per batch
HPC = H // HG         # heads per core = 4
DG = HPC * D          # 256: head-group width
SCALE = D ** -0.5

KT = C // 128         # 8 contraction tiles for qkv/x
NQ = N // 512         # 4 q-chunks
MT = N // 128         # 16 m (key) tiles


def _build_program():
    import concourse.bass as bass
    import concourse.bacc as bacc
    import concourse.tile as tile
    from concourse import mybir
    from contextlib import ExitStack

    f32 = mybir.dt.float32
    Exp = mybir.ActivationFunctionType.Exp
    f16 = mybir.dt.float16

    nc = bacc.Bacc()
    xT = nc.declare_dram_parameter("xT", [C, N], f16, isOutput=False)
    wq = nc.declare_dram_parameter("wq", [C, DG], f16, isOutput=False)
    wk = nc.declare_dram_parameter("wk", [C, DG], f16, isOutput=False)
    wv = nc.declare_dram_parameter("wv", [C, DG], f16, isOutput=False)
    wp = nc.declare_dram_parameter("wp", [DG, C], f16, isOutput=False)
    dmt = nc.declare_dram_parameter("dmt", [N, N], f16, isOutput=False)
    pout = nc.declare_dram_parameter("pout", [C, N], f16, isOutput=True)

    with tile.TileContext(nc) as tc, ExitStack() as ctx:
        big = ctx.enter_context(tc.tile_pool(name="big", bufs=1))
        epool = ctx.enter_context(tc.tile_pool(name="epool", bufs=6))
        small = ctx.enter_context(tc.tile_pool(name="small", bufs=2))
        outp = ctx.enter_context(tc.tile_pool(name="outp", bufs=4))
        # PSUM: psS slot [128,1024] x2 (4 banks) + pe0/pe1 (2) + pd0 (1) = 7 banks
        psS = ctx.enter_context(tc.tile_pool(name="psS", bufs=2, space="PSUM"))
        psE = ctx.enter_context(tc.tile_pool(name="psE", bufs=1, space="PSUM"))
        psD = ctx.enter_context(tc.tile_pool(name="psD", bufs=2, space="PSUM"))

        xt = big.tile([128, KT, N], f16)
        wq_s = big.tile([128, KT, DG], f16)
        wk_s = big.tile([128, KT, DG], f16)
        wv_s = big.tile([128, KT, DG], f16)
        qt = big.tile([128, 2, N], f16)
        kt = big.tile([128, 2, N], f16)
        vaug = big.tile([128, MT, HPC, D + 1], f16)
        vb = big.tile([128, MT, DG], f16)
        outT = big.tile([128, 2, N], f16)
        wp_s = big.tile([128, 2, C], f16)
        ones_sb = big.tile([128, MT * HPC], f32)
        ones16 = big.tile([1, D], f16)
        dms = big.tile([128, MT, N], f16)
        rscratch = nc.dram_tensor("rscratch", [8, 1024], f32)

        nc.vector.memset(ones_sb[:, :], 2.0)
        nc.vector.memset(ones16[:, :], 1.0)
        nc.vector.tensor_copy(vaug[:, :, :, D], ones_sb[:, :])

        for ct in range(KT):
            if ct == 0:
                nc.sync.dma_start(out=xt[:, 0, 0:1024], in_=xT[0:128, 0:1024])
                nc.sync.dma_start(out=xt[:, 0, 1024:2048], in_=xT[0:128, 1024:2048])
            else:
                nc.sync.dma_start(out=xt[:, ct, :], in_=xT[ct * 128:(ct + 1) * 128, :])
            nc.sync.dma_start(out=wk_s[:, ct, :], in_=wk[ct * 128:(ct + 1) * 128, :])
        for ct in range(KT):
            nc.sync.dma_start(out=wv_s[:, ct, :], in_=wv[ct * 128:(ct + 1) * 128, :])
        for ct in range(KT):
            nc.sync.dma_start(out=wq_s[:, ct, :], in_=wq[ct * 128:(ct + 1) * 128, :])
        for jo in range(2):
            nc.sync.dma_start(out=wp_s[:, jo, :], in_=wp[jo * 128:(jo + 1) * 128, :])
        for mt in range(MT):
            nc.sync.dma_start(out=dms[:, mt, :], in_=dmt[mt * 128:(mt + 1) * 128, :])

        # ---- phase 1: k^T first, then v, then q^T (attn consumers need k/v whole) ----
        def qk_proj(w_s, dst, scale, goff):
            for jo in range(2):
                for nq in range(NQ):
                    g = goff + jo * NQ + nq
                    ps = psS.tile([128, 512], f32, name="ps", tag="psS")
                    for i in range(KT):
                        ct = (g + i) % KT
                        nc.tensor.matmul(
                            ps[:, :],
                            lhsT=w_s[:, ct, jo * 128:(jo + 1) * 128],
                            rhs=xt[:, ct, nq * 512:(nq + 1) * 512],
                            start=(i == 0), stop=(i == KT - 1),
                        )
                    if scale != 1.0:
                        nc.vector.tensor_scalar_mul(
                            dst[:, jo, nq * 512:(nq + 1) * 512], ps[:, :], scale)
                    else:
                        nc.vector.tensor_copy(dst[:, jo, nq * 512:(nq + 1) * 512], ps[:, :])

        # k^T: first 6 output groups accumulate ct-outer across 6 PSUM slots so
        # each arriving xt tile feeds 6 matmuls (PE keeps pace with the DMA).
        kgroups = [(jo, nq) for jo in range(2) for nq in range(NQ)]
        ktags = ["psS", "psS", "pe0", "pe1", "pd0", "pd0"]
        kps = {}
        for i, g in enumerate(kgroups[:6]):
            if ktags[i] in ("pe0", "pe1"):
                kps[g] = psE.tile([128, 512], f32, name=f"kp{i}", tag=ktags[i])
            elif ktags[i] == "pd0":
                kps[g] = psD.tile([128, 512], f32, name=f"kp{i}", tag="pd0")
            else:
                kps[g] = psS.tile([128, 512], f32, name=f"kp{i}", tag="psS")
        for ct in range(KT):
            for jo, nq in kgroups[:6]:
                nc.tensor.matmul(
                    kps[(jo, nq)][:, :],
                    lhsT=wk_s[:, ct, jo * 128:(jo + 1) * 128],
                    rhs=xt[:, ct, nq * 512:(nq + 1) * 512],
                    start=(ct == 0), stop=(ct == KT - 1),
                )
        corder = sorted(range(6), key=lambda i: 0 if ktags[i] in ("pe0", "pe1") else 1)
        for i in corder:
            jo, nq = kgroups[i]
            nc.vector.tensor_copy(kt[:, jo, nq * 512:(nq + 1) * 512], kps[(jo, nq)][:, :])
        for jo, nq in kgroups[6:]:
            ps = psS.tile([128, 512], f32, name="ps", tag="psS")
            for i in range(KT):
                ct = (nq + i) % KT
                nc.tensor.matmul(
                    ps[:, :],
                    lhsT=wk_s[:, ct, jo * 128:(jo + 1) * 128],
                    rhs=xt[:, ct, nq * 512:(nq + 1) * 512],
                    start=(i == 0), stop=(i == KT - 1),
                )
            nc.vector.tensor_copy(kt[:, jo, nq * 512:(nq + 1) * 512], ps[:, :])

        for mt in range(MT):
            ps = psE.tile([128, DG], f32, name="ps", tag=f"pe{mt % 2}", padded_shape=[128, 512])
            for i in range(KT):
                ct = (mt + i) % KT
                nc.tensor.matmul(
                    ps[:, :],
                    lhsT=xt[:, ct, mt * 128:(mt + 1) * 128],
                    rhs=wv_s[:, ct, :],
                    start=(i == 0), stop=(i == KT - 1),
                )
            nc.vector.tensor_copy(vaug[:, mt, :, 0:D], ps[:, :])
            nc.vector.tensor_copy(vb[:, mt, :], ps[:, :])

        qk_proj(wq_s, qt, SCALE, 4)

        # ---- phase 2: attention, 2 heads (one k/q partition tile) per pass ----
        def proj_group(nq, co):
            qsl = slice(nq * 512, (nq + 1) * 512)
            ps = psD.tile([128, 512], f32, name="ps", tag="pd0")
            for jo in range(2):
                nc.tensor.matmul(
                    ps[:, :],
                    lhsT=wp_s[:, jo, co * 128:(co + 1) * 128],
                    rhs=outT[:, jo, qsl],
                    start=(jo == 0), stop=(jo == 1),
                )
            so = outp.tile([128, 512], f16)
            nc.vector.tensor_copy(so[:, :], ps[:, :])
            nc.sync.dma_start(out=pout[co * 128:(co + 1) * 128, qsl], in_=so[:, :])

        pending_proj = None
        for nq in range(NQ):
            qsl = slice(nq * 512, (nq + 1) * 512)
            for hp in range(2):
                pe0 = psE.tile([D + 1, 512], f32, name="pe0", tag="pe0")
                pe1 = psE.tile([D + 1, 512], f32, name="pe1", tag="pe1")
                pd = psD.tile([128, 512], f32, name="pd", tag="pd0")
                for mt in range(MT):
                    msl = slice(mt * 128, (mt + 1) * 128)
                    nc.tensor.matmul(
                        pd[:, :],
                        lhsT=vb[:, mt, hp * 128:(hp + 1) * 128],
                        rhs=dms[:, mt, qsl],
                        start=(mt == 0), stop=(mt == MT - 1),
                    )
                    sps = psS.tile([128, 1024], f32, name="sps", tag="psS")
                    nc.tensor.matmul(
                        sps[:, 0:512],
                        lhsT=kt[0:D, hp, msl], rhs=qt[0:D, hp, qsl],
                        start=True, stop=True,
                    )
                    nc.tensor.matmul(
                        sps[:, 512:1024],
                        lhsT=kt[D:2 * D, hp, msl], rhs=qt[D:2 * D, hp, qsl],
                        start=True, stop=True,
                    )
                    et = epool.tile([128, 1024], f16)
                    nc.scalar.activation(et[:, :], sps[:, :], Exp)
                    nc.tensor.matmul(
                        pe0[:, :], lhsT=vaug[:, mt, 2 * hp, :], rhs=et[:, 0:512],
                        start=(mt == 0), stop=(mt == MT - 1),
                    )
                    nc.tensor.matmul(
                        pe1[:, :], lhsT=vaug[:, mt, 2 * hp + 1, :], rhs=et[:, 512:1024],
                        start=(mt == 0), stop=(mt == MT - 1),
                    )
                    if pending_proj is not None and hp == 0 and 1 <= mt <= 8:
                        proj_group(pending_proj, mt - 1)
                # epilogue. Non-last chunks: free the PSUM banks with quick
                # copies, then normalize off the critical path (0.5/r broadcast
                # via DRAM bounce). Last chunk: nothing needs the banks again,
                # so read the accumulators directly and broadcast via a tiny
                # fp16 PE matmul (the PE is idle in the tail).
                slot = nq * 2 + hp
                last = (nq == NQ - 1 and hp == 1)
                if last:
                    pe_s0, pe_s1, pd_s = pe0, pe1, pd
                else:
                    pe_s0 = small.tile([D + 1, 512], f32, name="pe_s0", tag="pe_s0")
                    nc.vector.tensor_copy(pe_s0[:, :], pe0[:, :])
                    pe_s1 = small.tile([D + 1, 512], f32, name="pe_s1", tag="pe_s1")
                    nc.vector.tensor_copy(pe_s1[:, :], pe1[:, :])
                    pd_s = small.tile([128, 512], f32, name="pd_s", tag="pd_s")
                    nc.vector.tensor_copy(pd_s[:, :], pd[:, :])
                rec2 = small.tile([1, 1024], f16 if last else f32, name="rec2",
                                  tag="rec2l" if last else "rec2")
                for half, pes in ((0, pe_s0), (1, pe_s1)):
                    with nc.allow_low_precision(reason="0.5/r broadcast"):
                        nc.vector.reciprocal(
                            rec2[:, half * 512:(half + 1) * 512], pes[D:D + 1, :])
                if last:
                    bcp = psS.tile([D, 1024], f32, name="bcp", tag="psS",
                                   padded_shape=[128, 1024])
                    nc.tensor.matmul(bcp[:, 0:512], lhsT=ones16[:, :],
                                     rhs=rec2[:, 0:512], start=True, stop=True)
                    nc.tensor.matmul(bcp[:, 512:1024], lhsT=ones16[:, :],
                                     rhs=rec2[:, 512:1024], start=True, stop=True)
                    bcs = small.tile([D, 1024], f32, name="bcs", tag="bcs")
                    nc.vector.tensor_copy(bcs[:, :], bcp[:, :])
                else:
                    nc.sync.dma_start(out=rscratch[slot:slot + 1, :], in_=rec2[:, :])
                    row = rscratch[slot, :]
                    bc_ap = bass.AP(tensor=row.tensor, offset=row.offset,
                                    ap=[[0, D]] + list(row.ap))
                    bcs = small.tile([D, 1024], f32, name="bcs", tag="bcs")
                    nc.sync.dma_start(out=bcs[:, :], in_=bc_ap)
                for half, pes in ((0, pe_s0), (1, pe_s1)):
                    t1 = small.tile([128, 512], f32, name="t1", tag="t1")
                    nc.vector.tensor_mul(
                        t1[half * D:(half + 1) * D, :], pes[0:D, :],
                        bcs[:, half * 512:(half + 1) * 512])
                    nc.vector.tensor_add(
                        outT[half * D:(half + 1) * D, hp, qsl],
                        t1[half * D:(half + 1) * D, :],
                        pd_s[half * D:(half + 1) * D, :],
                    )
            pending_proj = nq
        for co in range(C // 128):
            proj_group(NQ - 1, co)
    nc.compile()
    return nc


_PROGRAM = None


def _get_program():
    global _PROGRAM
    if _PROGRAM is None:
        _PROGRAM = _build_program()
    return _PROGRAM


def _make_in_maps(x, distance_matrix, W_qkv, W_proj):
    in_maps = []
    for core in range(NCORES):
        b, hg = divmod(core, HG)
        sl = slice(hg * DG, (hg + 1) * DG)
        in_maps.append({
            "xT": np.ascontiguousarray(x[b].T).astype(np.float16),
            "wq": np.ascontiguousarray(W_qkv[:, sl]).astype(np.float16),
            "wk": np.ascontiguousarray(W_qkv[:, C + hg * DG:C + (hg + 1) * DG]).astype(np.float16),
            "wv": np.ascontiguousarray(W_qkv[:, 2 * C + hg * DG:2 * C + (hg + 1) * DG]).astype(np.float16),
            "wp": np.ascontiguousarray(W_proj[sl, :]).astype(np.float16),
            "dmt": np.ascontiguousarray(
                (0.5 * distance_matrix[b, 0].T).astype(np.float16)
            ),
        })
    return in_maps


def kernel(x, distance_matrix, W_qkv, W_proj, b_proj, _results_hook=None):
    from concourse.bass_utils import run_bass_kernel_spmd

    x = np.asarray(x)
    distance_matrix = np.asarray(distance_matrix)
    W_qkv = np.asarray(W_qkv)
    W_proj = np.asarray(W_proj)
    b_proj = np.asarray(b_proj)
    nc = _get_program()
    in_maps = _make_in_maps(x, distance_matrix, W_qkv, W_proj)
    res = run_bass_kernel_spmd(nc, in_maps, list(range(NCORES)))
    if _results_hook is not None:
        _results_hook(res)
    out = np.zeros((B, N, C), dtype=np.float32)
    for core in range(NCORES):
        b = core // HG
        out[b] += res.results[core]["pout"].T
    out += b_proj[None, None, :].astype(np.float32)
    return out



# revision 9
# speedup vs baseline: 1.0780x; 1.0780x over previous
"""Distributed attention kernel for Trainium2 (8 NeuronCores).

Reference computation (B=2, N=2048, C=1024, H=16, D=64, ALPHA=0.5):
    qkv = x @ W_qkv -> q,k,v [B,H,N,D]
    attn = softmax(q @ k^T / sqrt(D))
    attn = 0.5*dm + 0.5*attn
    out  = (attn @ v).reshape(B,N,C) @ W_proj + b_proj

Sharding: 8 cores = 2 batches x 4 head-groups (4 heads each).
Each core computes its head-group's slice end-to-end, including a partial
projection (row-slice of W_proj); host sums the 4 partials per batch.

Speed strategy vs the fp16 baseline: every matmul that tolerates it runs as
an fp8e4m3 DoubleRow matmul (0.5 PE cycles per output column AND 2 contraction
rows per partition), with hi+lo error compensation where fp8-single noise
would break the 2e-2 gate:
  - qkv projections: x and W split hi/lo fp8 (W pre-scaled x32 so W~N(0,1)
    quantizes in fp8 normal range); 3 accumulation terms hh+hl+lh give
    ~fp16 accuracy at 0.75x the fp16 PE cost with 256-deep contraction
    per instruction.
  - scores: stay fp16 (any fp8 on the q/k streams fails the error gate);
    q,k carry the x32*x32 weight scale, folded into the exp activation's
    scale argument (2^-13).
  - exp: ScalarE writes fp8e4m3 directly with bias -2 folded in (the bias
    cancels in softmax normalization; keeps e8 <= e^4.5 << 448).
  - attn@v: DoubleRow with lhsT slots = (v_hi, v_lo) and the fp8 exp stream
    duplicated across slots via a stride-0 AP (validated on HW): halves the
    e-stream PE cost while v stays at ~fp16 precision. The softmax
    denominator rides along as a 1/512 constant column (exact fp8
    subnormal) in the v_hi slot.
  - dm@v: DoubleRow with slots = m-tile pairs, hi/lo on both dm (host,
    pre-scaled x512) and v: 3 terms at 0.75x fp16 cost.
  - proj: DoubleRow over the jo row-pair dim, hi/lo on outT and W_proj
    (both x32-scaled); epilogue writes outT_hi/outT_lo fp8 directly.
All scale factors fold into existing instructions (activation scale/bias,
reciprocal ones-constant, pd path 512/32=16=lambda*alpha, pout copy 2^-10).
"""

import numpy as np

B, N, C, H, D = 2, 2048, 1024, 16, 64
NCORES = 8
HG = 4                # head-groups per batch
HPC = H // HG         # heads per core = 4
DG = HPC * D          # 256: head-group width
SCALE = D ** -0.5

CT = 4                # contraction pair-tiles for qkv (1024 = 4*256)
NQ = N // 512         # 4 q-chunks
MT = N // 128         # 16 m (key) tiles
PT = MT // 2          # 8 m pair-tiles

WS = 32.0             # weight scale (W_qkv, W_proj)
LAM = 32.0            # output scale carried by outT
DMSC = 512.0          # dm scale: dmS @ (v/32) = 16*dm@v = LAM*0.5*dm@v
ACT_SCALE = 0.125 / (WS * WS)   # 2^-13: undo 32*32 and apply D^-0.5
EBIAS = -2.0
ONEC = 1.0 / 512.0    # denominator column constant: exact fp8 subnormal


def _build_program():
    import concourse.bass as bass
    import concourse.bacc as bacc
    import concourse.tile as tile
    from concourse import mybir
    from contextlib import ExitStack

    f32 = mybir.dt.float32
    f16 = mybir.dt.float16
    f8 = mybir.dt.float8e4
    Exp = mybir.ActivationFunctionType.Exp
    DR = mybir.MatmulPerfMode.DoubleRow
    Alu = mybir.AluOpType

    nc = bacc.Bacc()
    xh = nc.declare_dram_parameter("xh", [128, CT, 2, N], f8, isOutput=False)
    xl = nc.declare_dram_parameter("xl", [128, CT, 2, N], f8, isOutput=False)
    wqh = nc.declare_dram_parameter("wqh", [128, CT, 2, DG], f8, isOutput=False)
    wql = nc.declare_dram_parameter("wql", [128, CT, 2, DG], f8, isOutput=False)
    wkh = nc.declare_dram_parameter("wkh", [128, CT, 2, DG], f8, isOutput=False)
    wkl = nc.declare_dram_parameter("wkl", [128, CT, 2, DG], f8, isOutput=False)
    wvh = nc.declare_dram_parameter("wvh", [128, CT, 2, DG], f8, isOutput=False)
    wvl = nc.declare_dram_parameter("wvl", [128, CT, 2, DG], f8, isOutput=False)
    wph = nc.declare_dram_parameter("wph", [128, 2, C], f8, isOutput=False)
    wpl = nc.declare_dram_parameter("wpl", [128, 2, C], f8, isOutput=False)
    dmh = nc.declare_dram_parameter("dmh", [128, PT, 2, N], f8, isOutput=False)
    dml = nc.declare_dram_parameter("dml", [128, PT, 2, N], f8, isOutput=False)
    pout = nc.declare_dram_parameter("pout", [C, N], f16, isOutput=True)

    with tile.TileContext(nc) as tc, ExitStack() as ctx:
        big = ctx.enter_context(tc.tile_pool(name="big", bufs=1))
        epool = ctx.enter_context(tc.tile_pool(name="epool", bufs=6))
        small = ctx.enter_context(tc.tile_pool(name="small", bufs=2))
        outp = ctx.enter_context(tc.tile_pool(name="outp", bufs=4))
        # PSUM: psS 2x[128,1024] (4 banks) + pe0/pe1 (2) + pd (1) + proj (1) = 8
        psS = ctx.enter_context(tc.tile_pool(name="psS", bufs=2, space="PSUM"))
        psE = ctx.enter_context(tc.tile_pool(name="psE", bufs=1, space="PSUM"))
        psD = ctx.enter_context(tc.tile_pool(name="psD", bufs=1, space="PSUM"))

        xh_s = big.tile([128, CT, 2, N], f8)
        xl_s = big.tile([128, CT, 2, N], f8)
        wq_s = {hl: big.tile([128, CT, 2, DG], f8, name=f"wq_s{hl}") for hl in range(2)}
        wk_s = {hl: big.tile([128, CT, 2, DG], f8, name=f"wk_s{hl}") for hl in range(2)}
        wv_s = {hl: big.tile([128, CT, 2, DG], f8, name=f"wv_s{hl}") for hl in range(2)}
        wp_s = {hl: big.tile([128, 2, C], f8, name=f"wp_s{hl}") for hl in range(2)}
        dm_s = {hl: big.tile([128, PT, 2, N], f8, name=f"dm_s{hl}") for hl in range(2)}
        qt = big.tile([128, 2, N], f16)
        kt = big.tile([128, 2, N], f16)
        # e@v stationary: [p, mt, head, (hi,lo), D + ones-col]
        vev = big.tile([128, MT, HPC, 2, D + 1], f8)
        # dm@v stationary: [p, (hi,lo), pt, m-parity, dg] (contiguous dg for
        # a mergeable 3-D [P, 2, M] DoubleRow lhsT view)
        vd = big.tile([128, 2, PT, 2, DG], f8)
        oth = big.tile([128, 2, N], f8)
        otl = big.tile([128, 2, N], f8)
        bias_t = big.tile([128, 1], f32)
        ones16 = big.tile([1, D], f16)
        rscratch = nc.dram_tensor("rscratch", [8, 1024], f32)

        nc.vector.memset(bias_t[:, :], EBIAS)
        nc.vector.memset(ones16[:, :], 1.0)
        nc.vector.memset(vev[:, :, :, 0, D], ONEC)
        nc.vector.memset(vev[:, :, :, 1, D], 0.0)

        # ---- input DMAs ----
        for cp in range(CT):
            nc.sync.dma_start(out=xh_s[:, cp, :, :], in_=xh[:, cp, :, :])
            nc.sync.dma_start(out=xl_s[:, cp, :, :], in_=xl[:, cp, :, :])
            if cp == 0:
                nc.sync.dma_start(out=wk_s[0][:, :, :, :], in_=wkh[:, :, :, :])
                nc.sync.dma_start(out=wk_s[1][:, :, :, :], in_=wkl[:, :, :, :])
        nc.sync.dma_start(out=wv_s[0][:, :, :, :], in_=wvh[:, :, :, :])
        nc.sync.dma_start(out=wv_s[1][:, :, :, :], in_=wvl[:, :, :, :])
        nc.sync.dma_start(out=wq_s[0][:, :, :, :], in_=wqh[:, :, :, :])
        nc.sync.dma_start(out=wq_s[1][:, :, :, :], in_=wql[:, :, :, :])
        nc.sync.dma_start(out=wp_s[0][:, :, :], in_=wph[:, :, :])
        nc.sync.dma_start(out=wp_s[1][:, :, :], in_=wpl[:, :, :])
        for pt in range(PT):
            nc.sync.dma_start(out=dm_s[0][:, pt, :, :], in_=dmh[:, pt, :, :])
            nc.sync.dma_start(out=dm_s[1][:, pt, :, :], in_=dml[:, pt, :, :])

        # 3 hi/lo accumulation terms: (x_hi,w_hi), (x_hi,w_lo), (x_lo,w_hi)
        def terms(w):
            return ((xh_s, w[0]), (xh_s, w[1]), (xl_s, w[0]))

        def qk_group(w, dst, jo, nq, ps):
            for cp in range(CT):
                for ti, (xs, ws) in enumerate(terms(w)):
                    nc.tensor.matmul(
                        ps[:, :],
                        lhsT=ws[:, cp, :, jo * 128:(jo + 1) * 128],
                        rhs=xs[:, cp, :, nq * 512:(nq + 1) * 512],
                        start=(cp == 0 and ti == 0), stop=(cp == CT - 1 and ti == 2),
                        perf_mode=DR,
                    )
            nc.vector.tensor_copy(dst[:, jo, nq * 512:(nq + 1) * 512], ps[:, :])

        # ---- phase 1: k first (ct-outer over 6 psum slots to overlap x DMA) ----
        kgroups = [(jo, nq) for jo in range(2) for nq in range(NQ)]
        ktags = ["psS", "psS", "pe0", "pe1", "pd0", "pd1"]
        kps = {}
        for i, g in enumerate(kgroups[:6]):
            if ktags[i] in ("pe0", "pe1"):
                kps[g] = psE.tile([128, 512], f32, name=f"kp{i}", tag=ktags[i])
            elif ktags[i].startswith("pd"):
                kps[g] = psD.tile([128, 512], f32, name=f"kp{i}", tag=ktags[i])
            else:
                kps[g] = psS.tile([128, 512], f32, name=f"kp{i}", tag="psS")
        for cp in range(CT):
            for ti in range(3):
                xs, ws = terms(wk_s)[ti]
                for jo, nq in kgroups[:6]:
                    nc.tensor.matmul(
                        kps[(jo, nq)][:, :],
                        lhsT=ws[:, cp, :, jo * 128:(jo + 1) * 128],
                        rhs=xs[:, cp, :, nq * 512:(nq + 1) * 512],
                        start=(cp == 0 and ti == 0), stop=(cp == CT - 1 and ti == 2),
                        perf_mode=DR,
                    )
        corder = sorted(range(6), key=lambda i: 0 if ktags[i] in ("pe0", "pe1") else 1)
        for i in corder:
            jo, nq = kgroups[i]
            nc.vector.tensor_copy(kt[:, jo, nq * 512:(nq + 1) * 512], kps[(jo, nq)][:, :])
        for jo, nq in kgroups[6:]:
            ps = psS.tile([128, 512], f32, name="ps", tag="psS")
            qk_group(wk_s, kt, jo, nq, ps)

        # ---- v: DR production + hi/lo fp8 epilogue straight into vev layout ----
        def v_tile(mt):
            msl = slice(mt * 128, (mt + 1) * 128)
            ps = psE.tile([128, DG], f32, name="ps", tag=f"pe{mt % 2}",
                          padded_shape=[128, 512])
            for cp in range(CT):
                for ti, (xs, ws) in enumerate(terms(wv_s)):
                    nc.tensor.matmul(
                        ps[:, :],
                        lhsT=xs[:, cp, :, msl],
                        rhs=ws[:, cp, :, :],
                        start=(cp == 0 and ti == 0), stop=(cp == CT - 1 and ti == 2),
                        perf_mode=DR,
                    )
            # v_hi = fp8(v/32); v_lo = fp8(v/32 - v_hi); written strided into vev
            psv = ps[:, :]
            ps4 = bass.AP(tensor=psv.tensor, offset=psv.offset,
                          ap=[list(psv.ap[0]), [D, HPC], [1, D]])
            hi = vev[:, mt, :, 0, 0:D]
            nc.vector.tensor_scalar_mul(hi, ps4, 1.0 / (WS * LAM))
            nc.vector.scalar_tensor_tensor(
                vev[:, mt, :, 1, 0:D], ps4, 1.0 / (WS * LAM), hi,
                op0=Alu.mult, op1=Alu.subtract,
            )
            # replicate (hi,lo) into the dm@v layout: SBUF->SBUF DMA per slot
            # (DMA APs are limited to 3 dims total)
            pt, par = divmod(mt, 2)
            for hl in range(2):
                nc.sync.dma_start(out=vd[:, hl, pt, par, :],
                                  in_=vev[:, mt, :, hl, 0:D])

        for mt in range(MT):
            v_tile(mt)

        # ---- q ----
        for jo in range(2):
            for nq in range(NQ):
                ps = psS.tile([128, 512], f32, name="ps", tag="psS")
                qk_group(wq_s, qt, jo, nq, ps)

        # ---- phase 2: attention ----
        def proj_group(nq, co):
            qsl = slice(nq * 512, (nq + 1) * 512)
            ps = psD.tile([128, 512], f32, name="pj", tag="pd1")
            for ti, (o_s, w_s) in enumerate(((oth, wp_s[0]), (otl, wp_s[0]), (oth, wp_s[1]))):
                nc.tensor.matmul(
                    ps[:, :],
                    lhsT=w_s[:, :, co * 128:(co + 1) * 128],
                    rhs=o_s[:, :, qsl],
                    start=(ti == 0), stop=(ti == 2),
                    perf_mode=DR,
                )
            so = outp.tile([128, 512], f16)
            nc.vector.tensor_scalar_mul(so[:, :], ps[:, :], 1.0 / (WS * LAM))
            nc.sync.dma_start(out=pout[co * 128:(co + 1) * 128, qsl], in_=so[:, :])

        pending_proj = None
        for nq in range(NQ):
            qsl = slice(nq * 512, (nq + 1) * 512)
            for hp in range(2):
                pe0 = psE.tile([D + 1, 512], f32, name="pe0", tag="pe0",
                               padded_shape=[128, 512])
                pe1 = psE.tile([D + 1, 512], f32, name="pe1", tag="pe1",
                               padded_shape=[128, 512])
                pd = psD.tile([128, 512], f32, name="pd", tag="pd0")
                pes = (pe0, pe1)
                for mt in range(MT):
                    msl = slice(mt * 128, (mt + 1) * 128)
                    if mt % 2 == 0:
                        pt = mt // 2
                        # dm@v: slots = m-tile pairs; 3 hi/lo terms
                        for ti, (vhl, dhl) in enumerate(((0, 0), (1, 0), (0, 1))):
                            nc.tensor.matmul(
                                pd[:, :],
                                lhsT=vd[:, vhl, pt, :, hp * 128:(hp + 1) * 128],
                                rhs=dm_s[dhl][:, pt, :, qsl],
                                start=(pt == 0 and ti == 0),
                                stop=(pt == PT - 1 and ti == 2),
                                perf_mode=DR,
                            )
                    sps = psS.tile([128, 1024], f32, name="sps", tag="psS")
                    nc.tensor.matmul(
                        sps[:, 0:512],
                        lhsT=kt[0:D, hp, msl], rhs=qt[0:D, hp, qsl],
                        start=True, stop=True,
                    )
                    nc.tensor.matmul(
                        sps[:, 512:1024],
                        lhsT=kt[D:2 * D, hp, msl], rhs=qt[D:2 * D, hp, qsl],
                        start=True, stop=True,
                    )
                    et = epool.tile([128, 1024], f8)
                    nc.scalar.activation(et[:, :], sps[:, :], Exp,
                                         bias=bias_t[:, :], scale=ACT_SCALE)
                    for hl in range(2):
                        r = et[:, hl * 512:(hl + 1) * 512]
                        rdup = bass.AP(tensor=r.tensor, offset=r.offset,
                                       ap=[list(r.ap[0]), [0, 2], list(r.ap[1])])
                        nc.tensor.matmul(
                            pes[hl][:, :],
                            lhsT=vev[:, mt, 2 * hp + hl, :, :],
                            rhs=rdup,
                            start=(mt == 0), stop=(mt == MT - 1),
                            perf_mode=DR,
                        )
                    if pending_proj is not None and hp == 0 and 1 <= mt <= 8:
                        proj_group(pending_proj, mt - 1)
                # epilogue: normalize softmax part, add dm part, emit outT hi/lo.
                slot = nq * 2 + hp
                last = (nq == NQ - 1 and hp == 1)
                if last:
                    pe_s0, pe_s1, pd_s = pe0, pe1, pd
                else:
                    pe_s0 = small.tile([D + 1, 512], f32, name="pe_s0", tag="pe_s0")
                    nc.vector.tensor_copy(pe_s0[:, :], pe0[:, :])
                    pe_s1 = small.tile([D + 1, 512], f32, name="pe_s1", tag="pe_s1")
                    nc.vector.tensor_copy(pe_s1[:, :], pe1[:, :])
                    pd_s = small.tile([128, 512], f32, name="pd_s", tag="pd_s")
                    nc.vector.tensor_copy(pd_s[:, :], pd[:, :])
                rec2 = small.tile([1, 1024], f16 if last else f32, name="rec2",
                                  tag="rec2l" if last else "rec2")
                for half, ps_ in ((0, pe_s0), (1, pe_s1)):
                    with nc.allow_low_precision(reason="1/r broadcast"):
                        nc.vector.reciprocal(
                            rec2[:, half * 512:(half + 1) * 512], ps_[D:D + 1, :])
                if last:
                    bcp = psS.tile([D, 1024], f32, name="bcp", tag="psS",
                                   padded_shape=[128, 1024])
                    nc.tensor.matmul(bcp[:, 0:512], lhsT=ones16[:, :],
                                     rhs=rec2[:, 0:512], start=True, stop=True)
                    nc.tensor.matmul(bcp[:, 512:1024], lhsT=ones16[:, :],
                                     rhs=rec2[:, 512:1024], start=True, stop=True)
                    bcs = small.tile([D, 1024], f32, name="bcs", tag="bcs")
                    nc.vector.tensor_copy(bcs[:, :], bcp[:, :])
                else:
                    nc.sync.dma_start(out=rscratch[slot:slot + 1, :], in_=rec2[:, :])
                    row = rscratch[slot, :]
                    bc_ap = bass.AP(tensor=row.tensor, offset=row.offset,
                                    ap=[[0, D]] + list(row.ap))
                    bcs = small.tile([D, 1024], f32, name="bcs", tag="bcs")
                    nc.sync.dma_start(out=bcs[:, :], in_=bc_ap)
                for half, ps_ in ((0, pe_s0), (1, pe_s1)):
                    hsl = slice(half * D, (half + 1) * D)
                    t1 = small.tile([128, 512], f32, name="t1", tag="t1")
                    nc.vector.tensor_mul(
                        t1[hsl, :], ps_[0:D, :], bcs[:, half * 512:(half + 1) * 512])
                    t2 = small.tile([128, 512], f16, name="t2", tag="t2")
                    nc.vector.tensor_add(t2[hsl, :], t1[hsl, :], pd_s[hsl, :])
                    hi = oth[hsl, hp, qsl]
                    nc.vector.tensor_copy(hi, t2[hsl, :])
                    nc.vector.scalar_tensor_tensor(
                        otl[hsl, hp, qsl], t2[hsl, :], 1.0, hi,
                        op0=Alu.mult, op1=Alu.subtract,
                    )
            pending_proj = nq
        for co in range(C // 128):
            proj_group(NQ - 1, co)
    nc.compile()
    return nc


_PROGRAM = None


def _get_program():
    global _PROGRAM
    if _PROGRAM is None:
        _PROGRAM = _build_program()
    return _PROGRAM


def _hilo(a, f8):
    hi = np.asarray(a, dtype=f8)
    lo = np.asarray(a - hi.astype(np.float32), dtype=f8)
    return hi, lo


def _pairct(a, nt):
    """[K, F] -> [128, nt, 2, F] with (p, t, i) <-> row t*256 + i*128 + p."""
    K, F = a.shape
    assert K == nt * 256
    return np.ascontiguousarray(a.reshape(nt, 2, 128, F).transpose(2, 0, 1, 3))


def _make_in_maps(x, distance_matrix, W_qkv, W_proj):
    import ml_dtypes
    f8 = ml_dtypes.float8_e4m3fn

    in_maps = []
    xTs = [np.ascontiguousarray(x[b].T).astype(np.float32) for b in range(B)]
    dmSs = [np.ascontiguousarray(DMSC * distance_matrix[b, 0].T).astype(np.float32)
            for b in range(B)]
    dm_pairs = []
    for b in range(B):
        dh, dl = _hilo(dmSs[b], f8)
        dm_pairs.append((_pairct(dh, PT), _pairct(dl, PT)))
    x_pairs = []
    for b in range(B):
        xhi, xlo = _hilo(xTs[b], f8)
        x_pairs.append((_pairct(xhi, CT), _pairct(xlo, CT)))

    for core in range(NCORES):
        b, hg = divmod(core, HG)
        sl = slice(hg * DG, (hg + 1) * DG)
        wq = WS * W_qkv[:, sl].astype(np.float32)
        wk = WS * W_qkv[:, C + hg * DG:C + (hg + 1) * DG].astype(np.float32)
        wv = WS * W_qkv[:, 2 * C + hg * DG:2 * C + (hg + 1) * DG].astype(np.float32)
        wp = WS * W_proj[sl, :].astype(np.float32)
        wqh_, wql_ = _hilo(wq, f8)
        wkh_, wkl_ = _hilo(wk, f8)
        wvh_, wvl_ = _hilo(wv, f8)
        wph_, wpl_ = _hilo(wp, f8)
        in_maps.append({
            "xh": x_pairs[b][0], "xl": x_pairs[b][1],
            "wqh": _pairct(wqh_, CT), "wql": _pairct(wql_, CT),
            "wkh": _pairct(wkh_, CT), "wkl": _pairct(wkl_, CT),
            "wvh": _pairct(wvh_, CT), "wvl": _pairct(wvl_, CT),
            "wph": np.ascontiguousarray(wph_.reshape(2, 128, C).transpose(1, 0, 2)),
            "wpl": np.ascontiguousarray(wpl_.reshape(2, 128, C).transpose(1, 0, 2)),
            "dmh": dm_pairs[b][0], "dml": dm_pairs[b][1],
        })
    return in_maps


def kernel(x, distance_matrix, W_qkv, W_proj, b_proj, _results_hook=None):
    from concourse.bass_utils import run_bass_kernel_spmd

    x = np.asarray(x)
    distance_matrix = np.asarray(distance_matrix)
    W_qkv = np.asarray(W_qkv)
    W_proj = np.asarray(W_proj)
    b_proj = np.asarray(b_proj)
    nc = _get_program()
    in_maps = _make_in_maps(x, distance_matrix, W_qkv, W_proj)
    res = run_bass_kernel_spmd(nc, in_maps, list(range(NCORES)))
    if _results_hook is not None:
        _results_hook(res)
    out = np.zeros((B, N, C), dtype=np.float32)
    for core in range(NCORES):
        b = core // HG
        out[b] += res.results[core]["pout"].T.astype(np.float32)
    out += b_proj[None, None, :].astype(np.float32)
    return out
